# revision 2
# baseline (speedup 1.0000x reference)
"""nn_NewsEncoder TRN2 kernel: 8-core data-parallel Bass/Tile implementation.

Self-contained: builds the Bass program once, shards the batch across 8
NeuronCores, runs via run_bass_kernel_spmd, and stitches the output.
"""

import numpy as np

from contextlib import ExitStack

import concourse.bass as bass
import concourse.tile as tile
from concourse import mybir
from concourse.masks import make_identity

F32 = mybir.dt.float32
BF16 = mybir.dt.bfloat16
I32 = mybir.dt.int32
AF = mybir.ActivationFunctionType
ALU = mybir.AluOpType
AX = mybir.AxisListType

VOCAB, D, S, H, HD, ATT = 50000, 300, 30, 20, 20, 200
D2 = H * HD  # 400
KC = [(0, 128), (128, 256), (256, 300)]  # chunks of the 300-dim feature axis
DC = [(i * 100, (i + 1) * 100) for i in range(4)]  # chunks of (h, hd) = 400
AC = [(0, 100), (100, 200)]  # chunks of ATT = 200
GB = 4          # batch elements per attention group (4 strips of 32)
NB = 16         # batch elements per block (Q^T-proj tile granularity)
NG = NB // GB   # groups per block


def strip_cols(ap, base, nb=GB, used=S, stride=32):
    """AP selecting `used` columns out of each 32-wide strip: [.., nb, used]."""
    return ap.rearrange("p (b j) -> p b j", j=stride)[:, base : base + nb, :used]


def build(ctx: ExitStack, tc: tile.TileContext, outs, ins, n_b: int):
    nc = tc.nc
    (z_out,) = outs
    x_idx = ins["x_idx"]      # [n_b*S] int32
    emb = ins["emb"]          # [VOCAB, D] f32
    pe = ins["pe"]            # [S, D] f32
    Wq = ins["Wq"]            # [H, D, D] f32
    bq = ins["bq"]            # [H, D] f32
    Wv = ins["Wv"]            # [H, HD, D] f32
    bv = ins["bv"]            # [H, HD] f32
    Wa = ins["Wa"]            # [ATT, D2] f32
    ba = ins["ba"]            # [ATT] f32
    wq2 = ins["wq2"]          # [1, ATT] f32
    bq2 = ins["bq2"]          # [1] f32

    R = n_b * S
    n_blk = n_b // NB
    assert n_b % NB == 0

    const = ctx.enter_context(tc.tile_pool(name="const", bufs=1))
    setup = ctx.enter_context(tc.tile_pool(name="setup", bufs=2))
    psum = ctx.enter_context(tc.tile_pool(name="psum", bufs=8, space="PSUM"))

    ident = const.tile([128, 128], BF16, name="ident")
    make_identity(nc, ident[:])
    ident_f = const.tile([128, 128], F32, name="ident_f")
    make_identity(nc, ident_f[:])
    ones_bf = const.tile([1, 128], BF16, name="ones_bf")
    nc.vector.memset(ones_bf[:], 1.0)

    # ---- weights: load + cast + PE-transpose into matmul layouts ----
    WqT = [const.tile([128, H * D], BF16, name=f"WqT{k}") for k in range(3)]
    WvT = [const.tile([128, D2], BF16, name=f"WvT{k}") for k in range(3)]
    WaT = [const.tile([100, ATT], BF16, name=f"WaT{d}") for d in range(4)]
    wq2T = [const.tile([100, 1], BF16, name=f"wq2T{a}") for a in range(2)]
    bq_sb = [const.tile([128, H], F32, name=f"bq{k}") for k in range(3)]
    bv_sb = [const.tile([100, 1], F32, name=f"bv{d}") for d in range(4)]
    ba_sb = [const.tile([100, 1], F32, name=f"ba{a}") for a in range(2)]
    bq2_sb = const.tile([1, 1], F32, name="bq2_sb")
    nc.sync.dma_start(bq2_sb[:], bq2[:, None])

    for h in range(H):
        for m0, m1 in KC:
            mn = m1 - m0
            wrow = setup.tile([128, D], F32, name="wrow", tag="wrow")
            nc.sync.dma_start(wrow[:mn, :], Wq[h, m0:m1, :])
            wbf = setup.tile([128, D], BF16, name="wbf", tag="wbf")
            nc.vector.tensor_copy(wbf[:mn, :], wrow[:mn, :])
            for k, (k0, k1) in enumerate(KC):
                kn = k1 - k0
                tp = psum.tile([128, 1024], BF16, name="tpsb", tag="ps")
                nc.tensor.transpose(tp[:kn, :mn], wbf[:mn, k0:k1], ident[:mn, :mn])
                nc.scalar.copy(WqT[k][:kn, h * D + m0 : h * D + m1], tp[:kn, :mn])
    for h in range(H):
        wrow = setup.tile([128, D], F32, name="wrow", tag="wrow")
        nc.sync.dma_start(wrow[:HD, :], Wv[h, :, :])
        wbf = setup.tile([128, D], BF16, name="wbf", tag="wbf")
        nc.vector.tensor_copy(wbf[:HD, :], wrow[:HD, :])
        for k, (k0, k1) in enumerate(KC):
            kn = k1 - k0
            tp = psum.tile([128, 1024], BF16, name="tpsb", tag="ps")
            nc.tensor.transpose(tp[:kn, :HD], wbf[:HD, k0:k1], ident[:HD, :HD])
            nc.scalar.copy(WvT[k][:kn, h * HD : (h + 1) * HD], tp[:kn, :HD])
    for ai, (a0, a1) in enumerate(AC):
        wrow = setup.tile([128, D2], F32, name="warow", tag="warow")
        nc.sync.dma_start(wrow[:100, :], Wa[a0:a1, :])
        wbf = setup.tile([128, D2], BF16, name="wabf", tag="wabf")
        nc.vector.tensor_copy(wbf[:100, :], wrow[:100, :])
        for di, (d0, d1) in enumerate(DC):
            tp = psum.tile([128, 1024], BF16, name="tpsb", tag="ps")
            nc.tensor.transpose(tp[:100, :100], wbf[:100, d0:d1], ident[:100, :100])
            nc.scalar.copy(WaT[di][:100, a0:a1], tp[:100, :100])
    w2row = setup.tile([1, ATT], F32, name="w2row")
    nc.sync.dma_start(w2row[:], wq2[:, :])
    w2bf = setup.tile([1, ATT], BF16, name="w2bf")
    nc.vector.tensor_copy(w2bf[:], w2row[:])
    for ai, (a0, a1) in enumerate(AC):
        tp = psum.tile([128, 1024], BF16, name="tpsb", tag="ps")
        nc.tensor.transpose(tp[:100, :1], w2bf[:1, a0:a1], ident[:1, :1])
        nc.scalar.copy(wq2T[ai][:100, :1], tp[:100, :1])
    brow = setup.tile([128, D], F32, name="brow")
    nc.sync.dma_start(brow[:H, :], bq[:, :])
    for k, (k0, k1) in enumerate(KC):
        kn = k1 - k0
        tpf = psum.tile([128, 512], F32, name="tpsf", tag="ps")
        nc.tensor.transpose(tpf[:kn, :H], brow[:H, k0:k1], ident_f[:H, :H])
        nc.scalar.copy(bq_sb[k][:kn, :H], tpf[:kn, :H])
    bv_flat = bv.rearrange("h d -> (h d)")
    for di, (d0, d1) in enumerate(DC):
        nc.sync.dma_start(bv_sb[di][:100, :], bv_flat[d0:d1, None])
    for ai, (a0, a1) in enumerate(AC):
        nc.sync.dma_start(ba_sb[ai][:100, :], ba[a0:a1, None])
    pe4 = const.tile([120, D], F32, name="pe4")
    for g in range(4):
        nc.sync.dma_start(pe4[g * S : (g + 1) * S, :], pe[:, :])
    n_grp = R // 120
    xind = const.tile([120, n_grp], I32, name="xind")
    nc.sync.dma_start(xind[:], x_idx.rearrange("(c p) -> p c", p=120))

    zt = [const.tile([100, n_b], F32, name=f"zt{d}") for d in range(4)]

    epool = ctx.enter_context(tc.tile_pool(name="epool", bufs=3))
    etp = ctx.enter_context(tc.tile_pool(name="etp", bufs=2))
    qtp = ctx.enter_context(tc.tile_pool(name="qtp", bufs=1))
    atp = ctx.enter_context(tc.tile_pool(name="atp", bufs=2))
    hpool = ctx.enter_context(tc.tile_pool(name="hpool", bufs=2))
    spool = ctx.enter_context(tc.tile_pool(name="spool", bufs=2))

    EW = NG * 128  # e^T tile width per block (strip layout)

    for blk in range(n_blk):
        # --- gather + transpose into strip-layout e^T tiles [128, EW] ---
        etile = [etp.tile([128, EW], BF16, name=f"eT{k}", tag=f"eT{k}") for k in range(3)]
        for k in range(3):
            # zero the 2 pad columns of every strip once per block
            nc.vector.memset(
                etile[k][:].rearrange("p (b j) -> p b j", j=32)[:, :, S:], 0.0
            )
        for g in range(NG):
            col = blk * NG + g
            erow = epool.tile([120, D], F32, name="erow", tag="erow")
            nc.scalar.copy(erow[:], pe4[:])
            nc.gpsimd.indirect_dma_start(
                out=erow[:],
                out_offset=None,
                in_=emb[:],
                in_offset=bass.IndirectOffsetOnAxis(ap=xind[:, col : col + 1], axis=0),
                compute_op=ALU.add,
            )
            ebf = epool.tile([120, D], BF16, name="ebf", tag="ebf")
            nc.vector.tensor_copy(ebf[:], erow[:])
            for k, (k0, k1) in enumerate(KC):
                kn = k1 - k0
                tp = psum.tile([128, 1024], BF16, name="tp", tag="ps")
                nc.tensor.transpose(tp[:kn, :120], ebf[:, k0:k1], ident[:120, :120])
                nc.scalar.copy(
                    strip_cols(etile[k][:kn, g * 128 : (g + 1) * 128], 0),
                    tp[:kn, :120].rearrange("p (b t) -> p b t", t=S),
                )

        # --- Q^T projection: qt[m] [o-chunk, (b, h, s)] bf16 (compact) ---
        qt = [qtp.tile([128, NB * H * S], BF16, name=f"qt{m}", tag=f"qt{m}")
              for m in range(3)]
        for h in range(H):
            for m, (m0, m1) in enumerate(KC):
                mn = m1 - m0
                qp = psum.tile([128, 512], F32, name="qp", tag="ps")
                for k, (k0, k1) in enumerate(KC):
                    kn = k1 - k0
                    nc.tensor.matmul(
                        qp[:mn, :EW],
                        lhsT=WqT[k][:kn, h * D + m0 : h * D + m1],
                        rhs=etile[k][:kn, :],
                        start=(k == 0),
                        stop=(k == 2),
                    )
                dst = qt[m][:mn, :].rearrange("p (b hs) -> p b hs", hs=H * S)[
                    :, :, h * S : (h + 1) * S
                ]
                src = qp[:mn, :EW].rearrange("p (b j) -> p b j", j=32)[:, :, :S]
                bias = bq_sb[m][:mn, h : h + 1]
                if (h + m) % 2 == 0:
                    nc.scalar.activation(dst, src, AF.Identity, bias=bias)
                else:
                    nc.vector.tensor_scalar_add(dst, src, bias)

        # --- attention per 4-batch group ---
        ht = [hpool.tile([100, NB * S], BF16, name=f"ht{d}", tag=f"ht{d}")
              for d in range(4)]
        for g in range(NG):
            ec = g * 128  # strip-column offset of this group inside etile
            at = atp.tile([128, H * S + 4], BF16, name="at", tag="at")
            nc.vector.memset(at[:, H * S :], 0.0)
            for half in range(2):
                hs0 = half * 300
                sp = psum.tile([128, 512], F32, name="sp", tag="ps")
                for b4 in range(GB):
                    bloc = g * GB + b4
                    for m, (m0, m1) in enumerate(KC):
                        mn = m1 - m0
                        nc.tensor.matmul(
                            sp[b4 * 32 : b4 * 32 + 32, :300],
                            lhsT=etile[m][:mn, ec + b4 * 32 : ec + b4 * 32 + 32],
                            rhs=qt[m][
                                :mn, bloc * H * S + hs0 : bloc * H * S + hs0 + 300
                            ],
                            start=(m == 0),
                            stop=(m == 2),
                            tile_position=(0, b4 * 32),
                        )
                ex = spool.tile([128, 300], F32, name="ex", tag="ex")
                nc.scalar.activation(ex[:], sp[:, :300], AF.Exp)
                sums = spool.tile([128, 10], F32, name="sums", tag="sums")
                nc.vector.tensor_reduce(
                    sums[:],
                    ex[:].rearrange("p (h s) -> p h s", s=S),
                    axis=AX.X,
                    op=ALU.add,
                )
                rs = spool.tile([128, 10], F32, name="rs", tag="rs")
                nc.vector.reciprocal(rs[:], sums[:])
                nc.vector.tensor_tensor(
                    at[:, hs0 : hs0 + 300].rearrange("p (h s) -> p h s", s=S),
                    ex[:].rearrange("p (h s) -> p h s", s=S),
                    rs[:].to_broadcast([128, 10, S]),
                    op=ALU.mult,
                )
            # v-projection: v [t(strips), (h,hd)=400]
            vp = psum.tile([128, 512], F32, name="vp", tag="ps")
            for k, (k0, k1) in enumerate(KC):
                kn = k1 - k0
                nc.tensor.matmul(
                    vp[:, :D2],
                    lhsT=etile[k][:kn, ec : ec + 128],
                    rhs=WvT[k][:kn, :],
                    start=(k == 0),
                    stop=(k == 2),
                )
            vt = spool.tile([128, D2], BF16, name="vt", tag="vt")
            nc.vector.tensor_copy(vt[:], vp[:, :D2])
            # hv row-major [s(strips), (h,hd)]
            hvp = psum.tile([128, 512], F32, name="hvp", tag="ps")
            for b4 in range(GB):
                sb = b4 * 32
                for h in range(H):
                    nc.tensor.matmul(
                        hvp[sb : sb + 32, h * HD : (h + 1) * HD],
                        lhsT=at[sb : sb + S, h * S : h * S + 32],
                        rhs=vt[sb : sb + S, h * HD : (h + 1) * HD],
                        start=True,
                        stop=True,
                        tile_position=(sb, sb),
                    )
            hvr = spool.tile([128, D2], BF16, name="hvr", tag="hvr")
            nc.vector.tensor_copy(hvr[:], hvp[:, :D2])
            # transpose to h^T [(h,hd)-chunk, (b,s)] with bv-bias drain
            for di, (d0, d1) in enumerate(DC):
                tph = psum.tile([128, 1024], BF16, name="tph", tag="ps")
                nc.tensor.transpose(tph[:100, :128], hvr[:, d0:d1], ident[:128, :128])
                nc.scalar.activation(
                    ht[di][:, g * 120 : (g + 1) * 120].rearrange(
                        "p (b t) -> p b t", t=S
                    ),
                    strip_cols(tph[:100, :128], 0),
                    AF.Identity,
                    bias=bv_sb[di][:100, :1],
                )

        # --- additive attention tail for the block ---
        th = [spool.tile([100, NB * S], BF16, name=f"th{a}", tag=f"th{a}")
              for a in range(2)]
        for ai, (a0, a1) in enumerate(AC):
            tpx = psum.tile([128, 512], F32, name="tailp", tag="ps")
            for di in range(4):
                nc.tensor.matmul(
                    tpx[:100, : NB * S],
                    lhsT=WaT[di][:100, a0:a1],
                    rhs=ht[di][:, :],
                    start=(di == 0),
                    stop=(di == 3),
                )
            nc.scalar.activation(th[ai][:], tpx[:100, : NB * S], AF.Tanh, bias=ba_sb[ai][:100, :1])
        apv = psum.tile([128, 512], F32, name="apv", tag="ps")
        for ai in range(2):
            nc.tensor.matmul(
                apv[:1, : NB * S], lhsT=wq2T[ai][:100, :1], rhs=th[ai][:, :],
                start=(ai == 0), stop=(ai == 1),
            )
        av = spool.tile([1, NB * S], BF16, name="av", tag="av")
        nc.vector.tensor_scalar_add(av[:], apv[:1, : NB * S], bq2_sb[:1, :1])
        ab = psum.tile([128, 512], F32, name="ab", tag="ps")
        nc.tensor.matmul(ab[:, : NB * S], lhsT=ones_bf[:1, :], rhs=av[:], start=True, stop=True)
        for di in range(4):
            hab = spool.tile([100, NB * S], BF16, name="hab", tag="hab")
            nc.vector.tensor_tensor(hab[:], ht[di][:, :], ab[:100, : NB * S], op=ALU.mult)
            nc.vector.tensor_reduce(
                zt[di][:, blk * NB : (blk + 1) * NB],
                hab[:].rearrange("p (b s) -> p b s", s=S),
                axis=AX.X,
                op=ALU.add,
            )

    # ---- z^T [400, n_b] -> z [n_b, 400] via PE transpose, then DMA out ----
    n_bc = (n_b + 127) // 128
    for bc in range(n_bc):
        c0 = bc * 128
        cn = min(128, n_b - c0)
        zsb = spool.tile([128, D2], F32, name="zsb", tag="zsb")
        for di in range(4):
            tpz = psum.tile([128, 512], F32, name="ztp", tag="ps")
            nc.tensor.transpose(
                tpz[:cn, :100], zt[di][:, c0 : c0 + cn], ident_f[:100, :100]
            )
            nc.scalar.copy(zsb[:cn, di * 100 : (di + 1) * 100], tpz[:cn, :100])
        nc.sync.dma_start(z_out[c0 : c0 + cn, :], zsb[:cn, :])


# ----------------------------------------------------------------------------
# SPMD driver
# ----------------------------------------------------------------------------
import concourse.bacc as bacc
from concourse.bass_utils import run_bass_kernel_spmd

B_FULL = 4096
N_CORES = 8
SHARD = B_FULL // N_CORES  # 512

_PROGRAM = None


def _build_program():
    global _PROGRAM
    if _PROGRAM is not None:
        return _PROGRAM
    nc = bacc.Bacc("TRN2", target_bir_lowering=False, debug=False)
    specs = {
        "x_idx": ([SHARD * S], I32),
        "emb": ([VOCAB, D], F32),
        "pe": ([S, D], F32),
        "Wq": ([H, D, D], F32),
        "bq": ([H, D], F32),
        "Wv": ([H, HD, D], F32),
        "bv": ([H, HD], F32),
        "Wa": ([ATT, D2], F32),
        "ba": ([ATT], F32),
        "wq2": ([1, ATT], F32),
        "bq2": ([1], F32),
    }
    ins = {
        name: nc.dram_tensor(name, shape, dt, kind="ExternalInput").ap()
        for name, (shape, dt) in specs.items()
    }
    z = nc.dram_tensor("z", [SHARD, D2], F32, kind="ExternalOutput").ap()
    with tile.TileContext(nc) as t, ExitStack() as _ctx:
        build(_ctx, t, [z], ins, n_b=SHARD)
    nc.compile()
    _PROGRAM = nc
    return nc


def kernel(x, emb, pe, Wq, bq, Wv, bv, Wa, ba, wq2, bq2):
    x = np.ascontiguousarray(np.asarray(x).astype(np.int32))
    shared = {
        "emb": np.ascontiguousarray(np.asarray(emb, np.float32)),
        "pe": np.ascontiguousarray(np.asarray(pe, np.float32)),
        "Wq": np.ascontiguousarray(np.asarray(Wq, np.float32)),
        "bq": np.ascontiguousarray(np.asarray(bq, np.float32)),
        "Wv": np.ascontiguousarray(np.asarray(Wv, np.float32)),
        "bv": np.ascontiguousarray(np.asarray(bv, np.float32)),
        "Wa": np.ascontiguousarray(np.asarray(Wa, np.float32)),
        "ba": np.ascontiguousarray(np.asarray(ba, np.float32)),
        "wq2": np.ascontiguousarray(np.asarray(wq2, np.float32)),
        "bq2": np.ascontiguousarray(np.asarray(bq2, np.float32).reshape(1)),
    }
    nc = _build_program()
    in_maps = [
        {"x_idx": x[c * SHARD : (c + 1) * SHARD].reshape(-1), **shared}
        for c in range(N_CORES)
    ]
    res = run_bass_kernel_spmd(nc, in_maps, list(range(N_CORES)))
    return np.concatenate([res.results[c]["z"] for c in range(N_CORES)], axis=0)
